# revision 6
# baseline (speedup 1.0000x reference)
"""Causal self-attention on 8 trn2 NeuronCores (bf16 datapath).

Problem: B=2, T=2048, C=1024, 16 heads of 64. Sharding: core = 4*b + g
(b = batch, g = head-group of 4 heads). Each core computes QKV projection
for its 4 heads, causal attention, and a partial c_proj (its 256 rows of
w_proj). Host sums the 4 partials per batch (the "all-reduce") + b_proj.

All SBUF operands are bf16 (PSUM accumulation stays fp32), which halves
input DMA bytes and removes the fp32r small-N matmul penalty, so the
S / AV matmuls can be trimmed to the causal boundary at 128-column
granularity.

Schedule: weights stream on the Activation HWDGE queue (wqk first),
x^T on the sync HWDGE queue; the pair-0 q/k projections consume x chunks
as they land, with zero-matmul warmups pinning the PE p-state ramp
through the DMA-paced stretch. Attention runs head-serial (one (pair,
half, h2) phase at a time), software-pipelined: S(t+1) and interleaved
filler (V tiles, pair-1 q/k quarter-projections, c_proj tiles) are
emitted before the AV work that parks on exp(t). AV runs flipped
(stationary P^T 128-col slice, moving V' -> out [tq, 65]) so each AV
matmul costs 65 PE cycles instead of ~512, and the softmax denominator
lands as a per-partition column: normalization is reciprocal +
tensor_scalar on DVE, then a PE transpose (identity moving operand)
rebuilds y^T for c_proj.

Per-core layouts (partition dim first):
  xT      (1024, 2048)  x[b]^T bf16; SBUF as 8 chunks (128, 2048)
  q^T/k^T (256, 2048)   bf16, 2 chunks each; chunk p = heads 2p, 2p+1;
                        1/sqrt(hs) folded into Wq,bq
  V'      (2048, 4, 65) bf16, natural + ones column (fused softmax
                        denominator: AV matmul emits [O^T; l] with M=65)
  S^T     (tk, tq) PSUM fp32; exp on ScalarE (no max subtraction: logits
                        ~N(0,1), exp cannot overflow); P^T bf16 in SBUF
  diag    strict-lower triangle of the diagonal 128x128 block is zeroed
                        by a bf16 upper-tri mask multiply on the DVE
  y^T     (256, 2048)   bf16 normalized attention out
  out     (2048, 1024)  bf16 partial y@w_proj; host sums in fp32
"""

import numpy as np
import ml_dtypes

import concourse.tile as tile
from concourse import bacc, mybir
from concourse.bass_utils import run_bass_kernel_spmd

B, T, C = 2, 2048, 1024
HS = 64
NCORES = 8
NHL = 4            # heads per core
TCH = 512          # tq / projection T chunk
NT = T // 128      # 16 tk tiles
F32 = mybir.dt.float32
BF16 = mybir.dt.bfloat16


def build_program():
    nc = bacc.Bacc("TRN2", target_bir_lowering=False, debug=False)

    xT_d = nc.dram_tensor("xT", [C, T], BF16, kind="ExternalInput").ap()
    wqk_d = nc.dram_tensor("wqk", [C, 512], BF16, kind="ExternalInput").ap()
    wv_d = nc.dram_tensor("wv", [C, 256], BF16, kind="ExternalInput").ap()
    wp_d = nc.dram_tensor("wp", [256, 1024], BF16, kind="ExternalInput").ap()
    bqk_d = nc.dram_tensor("bqk", [128, 4], F32, kind="ExternalInput").ap()
    bvb_d = nc.dram_tensor("bvb", [128, 320], BF16, kind="ExternalInput").ap()
    msk_d = nc.dram_tensor("msk", [128, 128], BF16, kind="ExternalInput").ap()
    idn_d = nc.dram_tensor("idn", [128, 128], BF16, kind="ExternalInput").ap()
    out_d = nc.dram_tensor("out", [T, C], BF16, kind="ExternalOutput").ap()

    with tile.TileContext(nc) as tc:
        _kernel(tc, out_d, xT_d, wqk_d, wv_d, wp_d, bqk_d, bvb_d, msk_d, idn_d)
    nc.compile()
    return nc


def _kernel(tc, out_d, xT_d, wqk_d, wv_d, wp_d, bqk_d, bvb_d, msk_d, idn_d):
    nc = tc.nc
    AF = mybir.ActivationFunctionType

    with (
        tc.tile_pool(name="persist", bufs=1) as pers,
        tc.tile_pool(name="ps", bufs=2, space="PSUM") as ps,
        tc.tile_pool(name="po", bufs=4, space="PSUM") as po,
    ):
        # Weights on the Activation HWDGE queue (bqk + wqk lead so the
        # first q-projection matmul only waits for wqk[0] + xt[0]); x^T
        # streams on the sync HWDGE queue in parallel.
        # zero operands for PE-warmup matmuls, memset first so the ramp
        # warmups can start before any DMA lands
        zs = pers.tile([64, 128], BF16, tag="zs")
        nc.vector.memset(zs[:], 0)
        zs2 = pers.tile([64, 512], BF16, tag="zs2")
        nc.vector.memset(zs2[:], 0)

        xp = tc.alloc_tile_pool(name="xp", bufs=1)
        xT3 = xT_d.rearrange("(c p) t -> c p t", p=128)
        wqk3 = wqk_d.rearrange("(c p) m -> c p m", p=128)
        wv3 = wv_d.rearrange("(c p) m -> c p m", p=128)
        wp3 = wp_d.rearrange("(c p) m -> c p m", p=128)
        xt, wqk, wv = [], [], []
        for c in range(8):
            w_ = pers.tile([128, 512], BF16, tag=f"wqk{c}", name=f"wqk{c}")
            nc.scalar.dma_start(out=w_, in_=wqk3[c])
            wqk.append(w_)
            t_ = xp.tile([128, T], BF16, tag=f"xt{c}", name=f"xt{c}")
            nc.sync.dma_start(out=t_, in_=xT3[c])
            xt.append(t_)
        bqk = pers.tile([128, 4], F32, tag="bqk")
        nc.scalar.dma_start(out=bqk, in_=bqk_d)
        for c in range(8):
            t_ = pers.tile([128, 256], BF16, tag=f"wv{c}", name=f"wv{c}")
            nc.scalar.dma_start(out=t_, in_=wv3[c])
            wv.append(t_)
        bvb = pers.tile([128, 320], BF16, tag="bvb")
        nc.scalar.dma_start(out=bvb, in_=bvb_d)
        msk = pers.tile([128, 128], BF16, tag="msk")
        nc.scalar.dma_start(out=msk, in_=msk_d)
        idn = pers.tile([128, 128], BF16, tag="idn")
        nc.scalar.dma_start(out=idn, in_=idn_d)
        wp = []
        for c in range(2):
            t_ = pers.tile([128, 1024], BF16, tag=f"wp{c}", name=f"wp{c}")
            nc.scalar.dma_start(out=t_, in_=wp3[c])
            wp.append(t_)

        def warm(pst, n=1, cols=512, first_start=False):
            for i in range(n):
                nc.tensor.matmul(
                    pst[0:128, 0:cols], zs[:], zs2[:, 0:cols],
                    start=(first_start and i == 0), stop=False,
                    skip_group_check=True,
                )

        # q^T / k^T chunks: m=0,1 -> q pairs, m=2,3 -> k pairs
        qk = [pers.tile([128, T], BF16, tag=f"qk{m}", name=f"qk{m}")
              for m in range(4)]
        # V' = [V | 1] per (tk-tile, head); ones column from bvb[:, 256:320]
        v_all = pers.tile([128, NT, NHL, HS + 1], BF16, tag="v_all",
                          name="v_all")
        nc.vector.tensor_copy(
            out=v_all[:, :, :, HS],
            in_=bvb[:, 256:320].rearrange("p (a b) -> p a b", a=NT),
        )
        # y^T chunks (normalized attention output), pair-stacked
        yt = [pers.tile([128, T], BF16, tag=f"yt{p}", name=f"yt{p}")
              for p in range(2)]

        # ---- QKV projection: qk[m] = (x @ wqk[:, m-chunk])^T + bias.
        # mq (po pool) and mk (ps pool) are emitted interleaved per x
        # chunk so the PE keeps pace with the x DMA stream. ----
        def qk_proj_pair(mq, mk, warmups=False):
            pq = [po.tile([128, TCH], F32, tag="po", name=f"pst{mq}_{i}")
                  for i in range(4)]
            wide = [ps.tile([128, 1024], F32, tag="st", name=f"pw{mk}_{i}")
                    for i in range(2)]
            pk = [wide[i // 2][:, TCH * (i % 2):TCH * (i % 2 + 1)]
                  for i in range(4)]
            for c in range(8):
                if warmups:
                    warm(pq[0][:], n=(8 if c == 0 else 0),
                         first_start=(c == 0))
                for m, pst in ((mq, pq), (mk, pk)):
                    lhsT = wqk[c][:, 128 * m:128 * (m + 1)]
                    for i in range(4):
                        nc.tensor.matmul(
                            pst[i][:],
                            lhsT,
                            xt[c][:, TCH * i:TCH * (i + 1)],
                            start=(c == 0),
                            stop=(c == 7),
                        )
            for m, pst in ((mq, pq), (mk, pk)):
                for i in range(4):
                    # split bias adds DVE/Pool; Act stays exp-only so the
                    # first attention exps are never queued behind adds
                    eng = nc.vector if i % 2 == 0 else nc.gpsimd
                    eng.tensor_scalar_add(
                        out=qk[m][:, TCH * i:TCH * (i + 1)],
                        in0=pst[i][:],
                        scalar1=bqk[:, m:m + 1],
                    )

        # ---- V tile t in natural layout (+bias) ----
        def v_tile(t):
            vp = po.tile([128, 256], F32, tag="po", name=f"vp{t}")
            for c in range(8):
                nc.tensor.matmul(
                    vp[:],
                    xt[c][:, 128 * t:128 * (t + 1)],
                    wv[c][:],
                    start=(c == 0),
                    stop=(c == 7),
                )
            nc.gpsimd.tensor_add(
                out=v_all[:, t, :, 0:HS],
                in0=vp[:].rearrange("p (h d) -> p h d", h=NHL),
                in1=bvb[:, 0:256].rearrange("p (h d) -> p h d", h=NHL),
            )

        # ---- fill queue: self-contained ~0.4-0.9us PE work units pumped
        # one per attention step, so the in-order PE queue always has
        # ready work behind the exp-gated AV matmuls ----
        fill_q = []

        def pump(k=1):
            for _ in range(k):
                if fill_q:
                    fill_q.pop(0)()

        # one 256-col slice of a late q/k projection chunk m: full c
        # accumulation in a single po slot
        def qk_slice(m, i):
            pst = po.tile([128, 256], F32, tag="po", name=f"qs{m}_{i}")
            for c in range(8):
                nc.tensor.matmul(
                    pst[:],
                    wqk[c][:, 128 * m:128 * (m + 1)],
                    xt[c][:, 256 * i:256 * (i + 1)],
                    start=(c == 0),
                    stop=(c == 7),
                )
            nc.gpsimd.tensor_scalar_add(
                out=qk[m][:, 256 * i:256 * (i + 1)],
                in0=pst[:],
                scalar1=bqk[:, m:m + 1],
            )

        # attention-side SBUF pools
        ptp = tc.alloc_tile_pool(name="pt", bufs=36)
        lrp = tc.alloc_tile_pool(name="lrec", bufs=8)
        ynp = tc.alloc_tile_pool(name="ynp", bufs=40)
        ostp = tc.alloc_tile_pool(name="ost", bufs=6)
        # y_norm2[pair][j]: [128 tq, 2 h2, 64 hs] bf16, filled by the two
        # h2 phases of a pair, then PE-transposed into yt[pair]
        yn = [[None] * NT for _ in range(2)]

        def drain_j(pair, h2, j, av, act_norm=False):
            # l sits at column 64 of the flipped AV output: per-partition,
            # so normalization is a reciprocal + per-partition-scale
            # multiply (on Act via Copy-with-scale when that engine has
            # slack, else tensor_scalar on DVE)
            if h2 == 0:
                yn[pair][j] = ynp.tile([128, 2, HS], BF16, tag="yn",
                                       name=f"yn{pair}{j}")
            lr = lrp.tile([128, 1], F32, tag="lr", name=f"lr{pair}{h2}{j}")
            nc.vector.reciprocal(out=lr[:], in_=av[:, HS:HS + 1])
            if act_norm:
                nc.scalar.activation(
                    out=yn[pair][j][:, h2, :],
                    in_=av[:, 0:HS],
                    func=AF.Copy,
                    bias=0.0,
                    scale=lr[:],
                )
            else:
                nc.vector.tensor_scalar_mul(
                    out=yn[pair][j][:, h2, :],
                    in0=av[:, 0:HS],
                    scalar1=lr[:],
                )
            if h2 == 1:
                pending_tp.append((pair, j))

        pending_tp = []

        def flush_tp():
            # transpose [128,128] into y^T via the PE (identity moving
            # operand), staged through PSUM; deferred a step so the PE
            # never parks waiting on the drain's DVE chain
            while pending_tp:
                pair, j = pending_tp.pop(0)
                tp = po.tile([128, 128], BF16, tag="po", name=f"tp{pair}{j}")
                nc.tensor.matmul(
                    tp[:],
                    yn[pair][j].rearrange("p a b -> p (a b)"),
                    idn[:],
                    start=True, stop=True, is_transpose=True,
                )
                nc.gpsimd.tensor_copy(
                    out=yt[pair][:, 128 * j:128 * (j + 1)], in_=tp[:])

        # ---- c_proj partial tile t: out rows [128t, 128t+128) ----
        stg_live = {}

        def proj_half(t, oc, tail=False):
            if oc == 0:
                stg_live[t] = ostp.tile([128, 1024], BF16, tag="stg",
                                        name=f"stg{t}")
            stg = stg_live[t]
            pp = po.tile([128, TCH], F32, tag="po", name=f"pp{t}{oc}")
            for p2 in range(2):
                nc.tensor.matmul(
                    pp[:],
                    yt[p2][:, 128 * t:128 * (t + 1)],
                    wp[p2][:, TCH * oc:TCH * (oc + 1)],
                    start=(p2 == 0),
                    stop=(p2 == 1),
                )
            # staging copies split DVE/Pool; Act stays exp-only
            eng = nc.vector if oc == 1 else nc.gpsimd
            eng.tensor_copy(
                out=stg[:, TCH * oc:TCH * (oc + 1)], in_=pp[:])
            if oc == 1:
                nc.sync.dma_start(out=out_d[128 * t:128 * (t + 1), :],
                                  in_=stg[:])
                del stg_live[t]

        def proj_tile(t, tail=False):
            proj_half(t, 0, tail)
            proj_half(t, 1, tail)

        # ---- attention phase for one (pair, half, h2): head-serial so
        # only 2 opr banks are live, leaving po slots for interleaved
        # proj/V work. extra(t) emits interleaved PE work after step t. ----
        def emit_S0(pair, half, h2):
            # first S tile of a phase, emitted inside the *previous* phase's
            # last step so the Act engine never starves across boundaries
            pb = 64 * h2
            st = ps.tile([128, 1024], F32, tag="st",
                         name=f"s0{pair}{half}{h2}")
            for cc in range(2):
                nc.tensor.matmul(
                    st[:, TCH * cc:TCH * (cc + 1)],
                    qk[2 + pair][pb:pb + 64, 0:128],
                    qk[pair][pb:pb + 64,
                             TCH * (2 * half + cc):TCH * (2 * half + cc + 1)],
                    start=True,
                    stop=True,
                )
            return st

        def attn(pair, half, h2, extra=None, st0=None, prelude=None, act_norm=False):
            t_end = 8 * (half + 1)
            h = 2 * pair + h2
            pb = 64 * h2
            pts = {}

            def emit_S(t):
                # row-packed K=64 matmul: head h2 lives at partitions
                # 64*h2..64*h2+64 of the qk chunks; moving dim trimmed
                # to the causal boundary (no small-N penalty in bf16).
                # The diagonal 128x128 block gets -30 accumulated onto its
                # strict-lower triangle (identity stationary x constant
                # mask), so exp masks causally with no DVE step.
                st = ps.tile([128, 1024], F32, tag="st",
                             name=f"st{pair}{half}{t}{h2}")
                for cc in range(2):
                    cg = 2 * half + cc
                    if cg < t // 4:
                        continue
                    sub0 = max(0, 128 * t - TCH * cg)
                    nc.tensor.matmul(
                        st[:, TCH * cc + sub0:TCH * (cc + 1)],
                        qk[2 + pair][pb:pb + 64, 128 * t:128 * (t + 1)],
                        qk[pair][pb:pb + 64, TCH * cg + sub0:TCH * (cg + 1)],
                        start=True,
                        stop=True,
                    )
                return st

            # software-pipelined emission: the PE queue is in-order, so
            # S(t+1) and any interleaved filler must be emitted BEFORE
            # AV(t), which parks waiting on exp(t).
            st = st0 if st0 is not None else emit_S(0)
            pre = None
            for t in range(t_end):
                rel = max(128 * t, 1024 * half) - 1024 * half
                pt = ptp.tile([128, 1024], BF16, tag="pt",
                              name=f"pt{pair}{half}{t}{h2}")
                nc.scalar.activation(
                    out=pt[:, rel:1024], in_=st[:, rel:1024], func=AF.Exp
                )
                if t + 1 < t_end:
                    st = emit_S(t + 1)
                elif prelude is not None:
                    pre = prelude()
                pump(1)
                flush_tp()
                if t // 8 == half:
                    # zero strict-lower triangle (tk > tq) of diag block
                    nc.vector.tensor_mul(
                        out=pt[:, rel:rel + 128],
                        in0=pt[:, rel:rel + 128],
                        in1=msk[:],
                    )
                pts[t] = pt
                # flipped AV, j-major burst: once exp(t) lands, the output
                # tile for tq-tile j == t is fully determined; accumulate
                # it over all pt(t' <= t) in one go. Stationary P^T 128-col
                # slice, moving V' [128, 65] -> out [tq, 65]: 65-cycle
                # matmuls, and l lands as a per-partition column.
                jj = t - 8 * half
                if jj >= 0:
                    avt = po.tile([128, HS + 1], F32, tag="po",
                                  name=f"av{pair}{half}{h2}{jj}")
                    for tp_ in range(t + 1):
                        nc.tensor.matmul(
                            avt[:],
                            pts[tp_][:, 128 * jj:128 * (jj + 1)],
                            v_all[:, tp_, h, :],
                            start=(tp_ == 0),
                            stop=(tp_ == t),
                        )
                    drain_j(pair, h2, t, avt, act_norm)
                if extra is not None:
                    extra(t)
            flush_tp()
            return pre

        def attn2(pair, half, extra=None, st0=None):
            t_end = 8 * (half + 1)
            h0 = 2 * pair
            pts = {}

            def emit_S2(t, h2):
                st = ps.tile([128, 1024], F32, tag="st",
                             name=f"s2{pair}{half}{t}{h2}")
                pb = 64 * h2
                for cc in range(2):
                    cg = 2 * half + cc
                    if cg < t // 4:
                        continue
                    sub0 = max(0, 128 * t - TCH * cg)
                    nc.tensor.matmul(
                        st[:, TCH * cc + sub0:TCH * (cc + 1)],
                        qk[2 + pair][pb:pb + 64, 128 * t:128 * (t + 1)],
                        qk[pair][pb:pb + 64, TCH * cg + sub0:TCH * (cg + 1)],
                        start=True,
                        stop=True,
                    )
                return st

            st2 = [st0 if st0 is not None else emit_S2(0, 0),
                   emit_S2(0, 1)]
            for t in range(t_end):
                rel = max(128 * t, 1024 * half) - 1024 * half
                for h2 in range(2):
                    pt = ptp.tile([128, 1024], BF16, tag="pt",
                                  name=f"p2{pair}{half}{t}{h2}")
                    nc.scalar.activation(
                        out=pt[:, rel:1024], in_=st2[h2][:, rel:1024],
                        func=AF.Exp,
                    )
                    if t + 1 < t_end:
                        st2[h2] = emit_S2(t + 1, h2)
                    pump(1)
                    if t // 8 == half:
                        nc.vector.tensor_mul(
                            out=pt[:, rel:rel + 128],
                            in0=pt[:, rel:rel + 128],
                            in1=msk[:],
                        )
                    pts[(t, h2)] = pt
                    jj = t - 8 * half
                    if jj >= 0:
                        avt = po.tile([128, HS + 1], F32, tag="po",
                                      name=f"a2{pair}{half}{h2}{jj}")
                        for tp_ in range(t + 1):
                            nc.tensor.matmul(
                                avt[:],
                                pts[(tp_, h2)][:, 128 * jj:128 * (jj + 1)],
                                v_all[:, tp_, h0 + h2, :],
                                start=(tp_ == 0),
                                stop=(tp_ == t),
                            )
                        drain_j(pair, h2, t, avt)
                    flush_tp()
                if extra is not None:
                    extra(t)

        # -------- schedule --------
        # Every attention phase is exp(Act)-bound; all remaining PE work
        # (V tiles, pair-1 q/k quarters, c_proj tiles) is interleaved into
        # those phases the moment its dependencies allow.
        def mk_extra(fns, at):
            sched = dict(zip(at, fns))
            return lambda t: sched[t]() if t in sched else None

        qk_proj_pair(0, 2, warmups=True)  # pair-0 q/k, paced by the x stream
        for t in range(3):
            v_tile(t)

        def mk_extra(fns, at):
            sched = dict(zip(at, fns))
            return lambda t: sched[t]() if t in sched else None

        # pair-0 half-0 attention; V tiles 3..15 stream through its exp
        # gaps three steps ahead of their first AV use
        s0 = attn(0, 0, 0,
                  extra=lambda t: v_tile(3 + t) if t < 8 else None,
                  prelude=lambda: emit_S0(0, 0, 1))
        s0 = attn(0, 0, 1,
                  extra=lambda t: v_tile(11 + t) if t < 5 else None,
                  st0=s0, prelude=lambda: emit_S0(0, 1, 0))

        # pair-0 half-1 attention; pair-1 q/k projection slices ride in
        # its exp gaps
        s0 = attn(0, 1, 0, extra=mk_extra(
            [lambda i=i: qk_slice(1, i) for i in range(4)] +
            [lambda i=i: qk_slice(3, i) for i in range(4)],
            [1, 3, 5, 7, 9, 11, 13, 15]),
            st0=s0, prelude=lambda: emit_S0(0, 1, 1))
        s0 = attn(0, 1, 1, extra=mk_extra(
            [lambda i=i: qk_slice(1, i) for i in range(4, 8)] +
            [lambda i=i: qk_slice(3, i) for i in range(4, 8)],
            [1, 3, 5, 7, 9, 11, 13, 15]),
            st0=s0, prelude=lambda: emit_S0(1, 0, 0))

        s0 = attn(1, 0, 0, st0=s0, prelude=lambda: emit_S0(1, 0, 1))
        # c_proj half-tiles interleave as soon as their yt columns are
        # complete (tiles 0-3 need only the first 512 tq columns)
        s0 = attn(1, 0, 1, extra=mk_extra(
            [lambda t=t, oc=oc: proj_half(t, oc)
             for t in (0, 1) for oc in (0, 1)], [4, 5, 6, 7]),
            st0=s0, prelude=lambda: emit_S0(1, 1, 0))
        p2sched = {k: [lambda t=2 + k, oc=oc: proj_half(t, oc)
                       for oc in (0, 1)] for k in range(6)}
        for k in range(6):
            # late merged-phase steps: exp is short there, so the oc=1
            # staging copy goes to the otherwise-idle Act engine
            p2sched[10 + k] = [lambda t=8 + k, oc=oc: proj_half(t, oc, True)
                               for oc in (0, 1)]
        p2sched[15] = p2sched[15] + [
            lambda oc=oc: proj_half(14, oc, True) for oc in (0, 1)]
        attn2(1, 1, extra=lambda t: [f() for f in p2sched.get(t, [])], st0=s0)
        proj_tile(15, tail=True)
        ostp.release()
        ynp.release()
        lrp.release()
        ptp.release()
        xp.release()


_PROG = None


def _get_program():
    global _PROG
    if _PROG is None:
        _PROG = build_program()
    return _PROG


def _bf(a):
    return np.ascontiguousarray(np.asarray(a, dtype=ml_dtypes.bfloat16))


def make_in_maps(x, w_attn, b_attn, w_proj, b_proj):
    x = np.asarray(x, dtype=np.float32)
    w_attn = np.asarray(w_attn, dtype=np.float32)
    b_attn = np.asarray(b_attn, dtype=np.float32)
    w_proj = np.asarray(w_proj, dtype=np.float32)
    s = 1.0 / np.sqrt(HS)
    wq, wk, wv = w_attn[:, 0:C], w_attn[:, C:2 * C], w_attn[:, 2 * C:3 * C]
    bq, bk, bv = b_attn[0:C], b_attn[C:2 * C], b_attn[2 * C:3 * C]
    # upper-triangular-inclusive causal mask for the S^T diagonal block
    msk = np.triu(np.ones((128, 128), dtype=np.float32))
    in_maps = []
    for core in range(NCORES):
        b, g = divmod(core, 4)
        cs = slice(256 * g, 256 * (g + 1))
        bqk_ = np.concatenate([bq[cs] * s, bk[cs]]).reshape(4, 128).T.copy()
        in_maps.append({
            "xT": _bf(x[b].T),
            "wqk": _bf(np.concatenate([wq[:, cs] * s, wk[:, cs]], axis=1)),
            "wv": _bf(wv[:, cs]),
            "wp": _bf(w_proj[cs, :]),
            "bqk": np.ascontiguousarray(bqk_),
            "bvb": _bf(np.concatenate([
                np.broadcast_to(bv[cs][None, :], (128, 256)),
                np.ones((128, 64), dtype=np.float32)], axis=1)),
            "msk": _bf(msk),
            "idn": _bf(np.eye(128, dtype=np.float32)),
        })
    return in_maps


def gather_output(results, b_proj):
    b_proj = np.asarray(b_proj, dtype=np.float32)
    out = np.empty((B, T, C), dtype=np.float32)
    for b in range(B):
        acc = results[4 * b]["out"].astype(np.float32)
        for g in range(1, 4):
            acc = acc + results[4 * b + g]["out"].astype(np.float32)
        out[b] = acc + b_proj[None, :]
    return out


def kernel(x, w_attn, b_attn, w_proj, b_proj):
    nc = _get_program()
    in_maps = make_in_maps(x, w_attn, b_attn, w_proj, b_proj)
    res = run_bass_kernel_spmd(nc, in_maps, core_ids=list(range(NCORES)))
    return gather_output(res.results, b_proj)



# revision 17
# speedup vs baseline: 1.0996x; 1.0996x over previous
"""Causal self-attention on 8 trn2 NeuronCores (fp8 DoubleRow datapath).

Problem: B=2, T=2048, C=1024, 16 heads of 64. Sharding: core = 4*b + g
(b = batch, g = head-group of 4 heads). Each core computes QKV projection
for its 4 heads, causal attention, and a partial c_proj (its 256 rows of
w_proj). Host sums the 4 partials per batch (the "all-reduce") + b_proj.

All big matmuls run as fp8e4m3 DoubleRow: operands carry a slot dim of 2
(two contraction k-tiles per instruction) at 0.5 PE cycles per output
column. Layouts are slot-paired end to end:

  xt[s]   (128, 2, 2048)  x^T, c-tile pair s (c = 256s + 128j + p)
  wqk[s]  (128, 2, 512)   w cols permuted to (m-chunk, head, dim%32);
                          m: 0=q-lo 1=q-hi 2=k-lo 3=k-hi
  qq/kk   (128, 2, 2048)  q^T/k^T: partition 32h+d = head h dim (32*slot+d)
  v_all   (128, 16, 4, 65) V'=[V|1] natural; slot pairs = adjacent tk tiles
  pt2     (128, 2, 1024)  P^T pairs: slot = tk-tile parity
  yt      (128, 2, 2048)  y^T: slot = head pair
  wp2     (128, 2, 1024)  w_proj rows: slot = head pair

S matmul per head h: lhsT = kk[32h:32h+32] (K=32, two 32-dim halves),
out = S^T [128 tk, tq]. Causal masking is a PE matmul: strict-upper x
(-240*I) accumulates -240 onto the diag block's strict-lower triangle
inside the S accumulation group, so exp (scale=1/8, applied there since
weights are unscaled for fp8 range) underflows masked entries to 0 and
no DVE mask step exists. AV runs flipped+paired: stationary P^T slice
pair, moving V' pair -> out [tq, 65] at 32.5 cycles per two tk tiles;
the softmax denominator lands as column 65 and normalization is
reciprocal + tensor_scalar on DVE, then a PE transpose rebuilds y^T.

Engines: Act does exp ONLY (it is the critical engine: ~58us of exp).
Pool (gpsimd) takes bias adds, tp->yt copies and half the proj staging
copies; DVE keeps the short drain chain + the other staging half.

Schedule: head-serial phases (h, half) x 8. QKV projection runs as 4
i-passes of 4 DR groups; passes 0-1 + v tiles 0-2 precede attention,
passes 2-3 and v tiles 3-15 ride phase (0,*) as fillers. c_proj rides
phase (3,*) as its yt columns complete; out DMA per 128-row tile.
"""

import numpy as np
import ml_dtypes

import concourse.tile as tile
from concourse import bacc, mybir
from concourse.bass_utils import run_bass_kernel_spmd

B, T, C = 2, 2048, 1024
HS = 64
NCORES = 8
NHL = 4            # heads per core
TCH = 512
NT = T // 128      # 16 tk tiles
F32 = mybir.dt.float32
BF16 = mybir.dt.bfloat16
F8 = mybir.dt.float8e4
E4 = ml_dtypes.float8_e4m3

# dtype knobs: fp8+DoubleRow per stage (False = bf16, two plain matmuls)
# (q/k and the S matmul stay bf16: matmul operand base partitions are
# restricted to {0,32,64}, which rules out the 4x32-partition fp8-DR S
# layout; with Act the bottleneck, bf16 S costs no total time anyway)
XW8 = True   # x / wqk / wv: QKV projection
PV8 = True   # P / V: AV matmul
YP8 = True   # y^T / wp: c_proj

DR = mybir.MatmulPerfMode.DoubleRow


def build_program():
    nc = bacc.Bacc("TRN2", target_bir_lowering=False, debug=False)

    xdt = F8 if XW8 else BF16
    vdt = F8 if PV8 else BF16
    ydt = F8 if YP8 else BF16

    x4_d = nc.dram_tensor("x4", [128, 4, 2, T], xdt, kind="ExternalInput").ap()
    wqk_d = nc.dram_tensor("wqk", [128, 4, 2, 512], xdt,
                           kind="ExternalInput").ap()
    wv_d = nc.dram_tensor("wv", [128, 4, 2, 256], xdt,
                          kind="ExternalInput").ap()
    wp_d = nc.dram_tensor("wp", [128, 2, 1024], ydt,
                          kind="ExternalInput").ap()
    bqk_d = nc.dram_tensor("bqk", [128, 4], F32, kind="ExternalInput").ap()
    bvb_d = nc.dram_tensor("bvb", [128, 320], BF16, kind="ExternalInput").ap()
    um_d = nc.dram_tensor("umsk", [128, 128], BF16, kind="ExternalInput").ap()
    ni_d = nc.dram_tensor("nid", [128, 128], BF16, kind="ExternalInput").ap()
    idn_d = nc.dram_tensor("idn", [128, 128], BF16, kind="ExternalInput").ap()
    out_d = nc.dram_tensor("out", [T, C], BF16, kind="ExternalOutput").ap()

    with tile.TileContext(nc) as tc:
        _kernel(tc, out_d, x4_d, wqk_d, wv_d, wp_d, bqk_d, bvb_d, um_d,
                ni_d, idn_d, xdt, vdt, ydt)
    nc.compile()
    return nc


def _kernel(tc, out_d, x4_d, wqk_d, wv_d, wp_d, bqk_d, bvb_d, um_d, ni_d,
            idn_d, xdt, vdt, ydt):
    nc = tc.nc
    AF = mybir.ActivationFunctionType

    def mm2(out, lhsT, rhs, start, stop, f8):
        # slot-paired matmul: lhsT/rhs [p, 2, *]. One DoubleRow fp8
        # instruction, or two plain matmuls over the slots.
        if f8:
            nc.tensor.matmul(out, lhsT, rhs, start=start, stop=stop,
                             perf_mode=DR)
        else:
            nc.tensor.matmul(out, lhsT[:, 0], rhs[:, 0], start=start,
                             stop=False)
            nc.tensor.matmul(out, lhsT[:, 1], rhs[:, 1], start=False,
                             stop=stop)

    with (
        tc.tile_pool(name="persist", bufs=1) as pers,
        tc.tile_pool(name="ps", bufs=2, space="PSUM") as ps,
        tc.tile_pool(name="po", bufs=4, space="PSUM") as po,
    ):
        # zero operands for PE-warmup matmuls; memset first so the ramp
        # warmups start before any DMA lands
        zs = pers.tile([64, 128], BF16, tag="zs")
        nc.vector.memset(zs[:], 0)
        zs2 = pers.tile([64, 512], BF16, tag="zs2")
        nc.vector.memset(zs2[:], 0)

        xp = tc.alloc_tile_pool(name="xp", bufs=1)
        xt = []
        for s in range(4):
            t_ = xp.tile([128, 2, T], xdt, tag=f"xt{s}", name=f"xt{s}")
            # two sub-DMAs per c-tile pair so consumption paces the stream
            nc.sync.dma_start(out=t_[:, 0, :], in_=x4_d[:, s, 0])
            nc.sync.dma_start(out=t_[:, 1, :], in_=x4_d[:, s, 1])
            xt.append(t_)
        # weight DMAs split across the (otherwise idle at t=0) Act and DVE
        # queues; Act's finish well before the first exp is enqueued
        wqk = []
        for s in range(4):
            w_ = pers.tile([128, 2, 512], xdt, tag=f"wqk{s}", name=f"wqk{s}")
            nc.scalar.dma_start(out=w_, in_=wqk_d[:, s])
            wqk.append(w_)
        bqk = pers.tile([128, 4], F32, tag="bqk")
        nc.scalar.dma_start(out=bqk, in_=bqk_d)
        umsk = pers.tile([128, 128], BF16, tag="umsk")
        nc.scalar.dma_start(out=umsk, in_=um_d)
        nid = pers.tile([128, 128], BF16, tag="nid")
        nc.scalar.dma_start(out=nid, in_=ni_d)
        wv = []
        for s in range(4):
            t_ = pers.tile([128, 2, 256], xdt, tag=f"wv{s}", name=f"wv{s}")
            nc.sync.dma_start(out=t_, in_=wv_d[:, s])
            wv.append(t_)
        bvb = pers.tile([128, 320], BF16, tag="bvb")
        nc.sync.dma_start(out=bvb, in_=bvb_d)
        idn = pers.tile([128, 128], BF16, tag="idn")
        nc.scalar.dma_start(out=idn, in_=idn_d)
        wp2 = pers.tile([128, 2, 1024], ydt, tag="wp2")
        nc.scalar.dma_start(out=wp2, in_=wp_d)

        # q^T / k^T chunks: m 0/1 -> q head-pairs, 2/3 -> k head-pairs;
        # chunk m partitions = heads (2p, 2p+1) x 64 dims. bf16: the S
        # matmul can't use fp8-DR (operand base-partition limits).
        qk = [pers.tile([128, T], BF16, tag=f"qk{m}", name=f"qk{m}")
              for m in range(4)]
        # V' = [V | 1] per (tk-tile, head); ones col from bvb[:, 256:320]
        v_all = pers.tile([128, NT, NHL, HS + 1], vdt, tag="v_all",
                          name="v_all")
        nc.vector.tensor_copy(
            out=v_all[:, :, :, HS],
            in_=bvb[:, 256:320].rearrange("p (a b) -> p a b", a=NT),
        )
        # y^T, slot = head pair
        yt = pers.tile([128, 2, T], ydt, tag="yt", name="yt")

        def warm(pst, n):
            for i in range(n):
                nc.tensor.matmul(
                    pst[0:128, 0:512], zs[:], zs2[:],
                    start=(i == 0), stop=False, skip_group_check=True,
                )

        # ---- QKV projection: 4 i-passes x 4 DR groups (m = chunk) ----
        def qk_group(m, i, pg):
            eng = nc.vector if (m + i) % 2 == 0 else nc.gpsimd
            eng.tensor_scalar_add(
                out=qk[m][:, TCH * i:TCH * (i + 1)],
                in0=pg[:],
                scalar1=bqk[:, m:m + 1],
            )

        def qk_pass(i, warmups=False):
            # 3 interleaved groups (po rotation) + the 4th after
            pgs = [po.tile([128, TCH], F32, tag="po", name=f"pg{m}_{i}")
                   for m in range(3)]
            if warmups:
                warm(pgs[0], 8)
            for s in range(4):
                for m in range(3):
                    mm2(pgs[m][:], wqk[s][:, :, 128 * m:128 * (m + 1)],
                        xt[s][:, :, TCH * i:TCH * (i + 1)],
                        s == 0, s == 3, XW8)
            for m in range(3):
                qk_group(m, i, pgs[m])
            pg3 = po.tile([128, TCH], F32, tag="po", name=f"pg3_{i}")
            for s in range(4):
                mm2(pg3[:], wqk[s][:, :, 384:512],
                    xt[s][:, :, TCH * i:TCH * (i + 1)], s == 0, s == 3, XW8)
            qk_group(3, i, pg3)

        def qk_sub(m, i):
            # one filler unit: a single (m, i) group
            pg = po.tile([128, TCH], F32, tag="po", name=f"pq{m}_{i}")
            for s in range(4):
                mm2(pg[:], wqk[s][:, :, 128 * m:128 * (m + 1)],
                    xt[s][:, :, TCH * i:TCH * (i + 1)], s == 0, s == 3, XW8)
            qk_group(m, i, pg)

        # ---- V tile t (natural layout, +bias) ----
        def v_tile(t):
            vp = po.tile([128, 256], F32, tag="po", name=f"vp{t}")
            for s in range(4):
                mm2(vp[:], xt[s][:, :, 128 * t:128 * (t + 1)], wv[s][:],
                    s == 0, s == 3, XW8)
            nc.gpsimd.tensor_add(
                out=v_all[:, t, :, 0:HS],
                in0=vp[:].rearrange("p (h d) -> p h d", h=NHL),
                in1=bvb[:, 0:256].rearrange("p (h d) -> p h d", h=NHL),
            )

        # ---- attention-side SBUF pools ----
        ptp = tc.alloc_tile_pool(name="pt", bufs=14)
        lrp = tc.alloc_tile_pool(name="lrec", bufs=8)
        ynp = tc.alloc_tile_pool(name="ynp", bufs=40)
        ostp = tc.alloc_tile_pool(name="ost", bufs=4)
        yn = [[None] * NT for _ in range(2)]
        pending_tp = []

        def drain_j(pair, h2, j, av):
            if h2 == 0:
                yn[pair][j] = ynp.tile([128, 2, HS], BF16, tag="yn",
                                       name=f"yn{pair}{j}")
            lr = lrp.tile([128, 1], F32, tag="lr", name=f"lr{pair}{h2}{j}")
            nc.vector.reciprocal(out=lr[:], in_=av[:, HS:HS + 1])
            nc.vector.tensor_scalar_mul(
                out=yn[pair][j][:, h2, :],
                in0=av[:, 0:HS],
                scalar1=lr[:],
            )
            if h2 == 1:
                pending_tp.append((pair, j))

        def flush_tp():
            while pending_tp:
                pair, j = pending_tp.pop(0)
                tp = po.tile([128, 128], BF16, tag="po", name=f"tp{pair}{j}")
                nc.tensor.matmul(
                    tp[:],
                    yn[pair][j].rearrange("p a b -> p (a b)"),
                    idn[:],
                    start=True, stop=True, is_transpose=True,
                )
                nc.gpsimd.tensor_copy(
                    out=yt[:, pair, 128 * j:128 * (j + 1)], in_=tp[:])

        # ---- c_proj partial: one DR matmul per (tile, out-col half) ----
        stg_live = {}

        def proj_half(t, oc, act_tail=False):
            if oc == 0:
                stg_live[t] = ostp.tile([128, 1024], BF16, tag="stg",
                                        name=f"stg{t}")
            stg = stg_live[t]
            pp = po.tile([128, TCH], F32, tag="po", name=f"pp{t}{oc}")
            mm2(pp[:], yt[:, :, 128 * t:128 * (t + 1)],
                wp2[:, :, TCH * oc:TCH * (oc + 1)], True, True, YP8)
            if act_tail:
                # Act is idle after its last exp; absorb tail staging there
                nc.scalar.copy(out=stg[:, TCH * oc:TCH * (oc + 1)], in_=pp[:])
            else:
                eng = nc.vector if oc == 1 else nc.gpsimd
                eng.tensor_copy(
                    out=stg[:, TCH * oc:TCH * (oc + 1)], in_=pp[:])
            if oc == 1:
                nc.sync.dma_start(out=out_d[128 * t:128 * (t + 1), :],
                                  in_=stg[:])
                del stg_live[t]

        # ---- S tile: row-packed K=64 bf16 matmuls (head h2 at partitions
        # 64*h2 of the pair chunks); the causal mask is folded into the
        # accumulation group as a -240 strict-lower add on the diagonal
        # block, so exp underflows masked entries to zero ----
        def emit_S(h, half, t):
            pair, h2 = divmod(h, 2)
            pb = 64 * h2
            st = ps.tile([128, 1024], F32, tag="st", name=f"st{h}{half}{t}")
            kb = qk[2 + pair][pb:pb + 64, 128 * t:128 * (t + 1)]
            diag = (t // 8) == half
            lo = (128 * t - 1024 * half) if diag else 0
            a = lo
            if diag:
                nc.tensor.matmul(
                    st[:, a:a + 128], kb,
                    qk[pair][pb:pb + 64,
                             1024 * half + a:1024 * half + a + 128],
                    start=True, stop=False)
                nc.tensor.matmul(st[:, a:a + 128], umsk[:], nid[:],
                                 start=False, stop=True)
                a += 128
            while a < 1024:
                b = min(a + TCH, 1024)
                nc.tensor.matmul(
                    st[:, a:b], kb,
                    qk[pair][pb:pb + 64, 1024 * half + a:1024 * half + b],
                    start=True, stop=True)
                a = b
            return st

        # ---- AV for out-tile j (block jj in this half): DR over adjacent
        # tk-tile pairs, odd tail as a plain matmul ----
        def emit_AV(h, t, jj, pts2):
            avt = po.tile([128, HS + 1], F32, tag="po",
                          name=f"av{h}{t}")
            n = t + 1
            np2 = n // 2
            for s2 in range(np2):
                mm2(avt[:], pts2[s2][:, :, 128 * jj:128 * (jj + 1)],
                    v_all[:, 2 * s2:2 * s2 + 2, h, :],
                    s2 == 0, (s2 == np2 - 1) and (n % 2 == 0), PV8)
            if n % 2:
                nc.tensor.matmul(
                    avt[:],
                    pts2[n // 2][:, 0, 128 * jj:128 * (jj + 1)],
                    v_all[:, n - 1, h, :],
                    start=(np2 == 0), stop=True)
            return avt

        pdt = F8 if PV8 else BF16

        def attn(h, half, extra=None, st0=None, prelude=None):
            pair, h2 = divmod(h, 2)
            t_end = 8 * (half + 1)
            pts2 = {}
            st = st0 if st0 is not None else emit_S(h, half, 0)
            pre = None
            for t in range(t_end):
                diag = (t // 8) == half
                lo = (128 * t - 1024 * half) if diag else 0
                s2, par = divmod(t, 2)
                if par == 0:
                    pts2[s2] = ptp.tile([128, 2, 1024], pdt, tag="pt",
                                        name=f"pt{h}{half}{s2}")
                nc.scalar.activation(
                    out=pts2[s2][:, par, lo:1024], in_=st[:, lo:1024],
                    func=AF.Exp, scale=0.125,
                )
                if t + 1 < t_end:
                    st = emit_S(h, half, t + 1)
                elif prelude is not None:
                    pre = prelude()
                flush_tp()
                jj = t - 8 * half
                if jj >= 0:
                    avt = emit_AV(h, t, jj, pts2)
                    drain_j(pair, h2, t, avt)
                if extra is not None:
                    extra(t)
            flush_tp()
            return pre

        def mk_extra(sched):
            return lambda t: [f() for f in sched.get(t, [])]

        # -------- schedule --------
        qk_pass(0, warmups=True)
        qk_pass(1)
        for t in range(3):
            v_tile(t)

        # phase (0,0): projection passes 2-3 and v tiles 3-7 as fillers
        e00 = {
            0: [lambda: qk_sub(0, 2), lambda: qk_sub(1, 2)],
            1: [lambda: qk_sub(2, 2), lambda: qk_sub(3, 2)],
            2: [lambda: qk_sub(0, 3), lambda: v_tile(3)],
            3: [lambda: qk_sub(1, 3), lambda: v_tile(4)],
            4: [lambda: qk_sub(2, 3), lambda: v_tile(5)],
            5: [lambda: qk_sub(3, 3), lambda: v_tile(6)],
            6: [lambda: v_tile(7)],
        }
        s0 = attn(0, 0, extra=mk_extra(e00),
                  prelude=lambda: emit_S(0, 1, 0))
        e01 = {t: [lambda tt=8 + t: v_tile(tt)] for t in range(8)}
        s0 = attn(0, 1, st0=s0, extra=mk_extra(e01),
                  prelude=lambda: emit_S(1, 0, 0))
        s0 = attn(1, 0, st0=s0, prelude=lambda: emit_S(1, 1, 0))
        s0 = attn(1, 1, st0=s0, prelude=lambda: emit_S(2, 0, 0))
        s0 = attn(2, 0, st0=s0, prelude=lambda: emit_S(2, 1, 0))
        s0 = attn(2, 1, st0=s0, prelude=lambda: emit_S(3, 0, 0))
        # phase (3,0): proj tiles 0-5 as their yt columns complete
        e30 = {t: [lambda tt=t - 2, oc=oc: proj_half(tt, oc)
                   for oc in (0, 1)] for t in range(2, 8)}
        s0 = attn(3, 0, st0=s0, extra=mk_extra(e30),
                  prelude=lambda: emit_S(3, 1, 0))
        # phase (3,1): tiles 6-7 up front, 8-13 as drains land
        e31 = {0: [lambda oc=oc: proj_half(6, oc) for oc in (0, 1)],
               1: [lambda oc=oc: proj_half(7, oc) for oc in (0, 1)]}
        for k in range(6):
            e31[10 + k] = [lambda tt=8 + k, oc=oc: proj_half(tt, oc)
                           for oc in (0, 1)]
        attn(3, 1, st0=s0, extra=mk_extra(e31))
        for t in (14, 15):
            proj_half(t, 0, act_tail=True)
            proj_half(t, 1, act_tail=True)
        ostp.release()
        ynp.release()
        lrp.release()
        ptp.release()
        xp.release()


_PROG = None


def _get_program():
    global _PROG
    if _PROG is None:
        _PROG = build_program()
    return _PROG


def _bf(a):
    return np.ascontiguousarray(np.asarray(a, dtype=ml_dtypes.bfloat16))


def _cast(a, f8):
    dt = E4 if f8 else ml_dtypes.bfloat16
    return np.ascontiguousarray(np.asarray(a, dtype=dt))


def make_in_maps(x, w_attn, b_attn, w_proj, b_proj):
    x = np.asarray(x, dtype=np.float32)
    w_attn = np.asarray(w_attn, dtype=np.float32)
    b_attn = np.asarray(b_attn, dtype=np.float32)
    w_proj = np.asarray(w_proj, dtype=np.float32)
    wq, wk, wv = w_attn[:, 0:C], w_attn[:, C:2 * C], w_attn[:, 2 * C:3 * C]
    bq, bk, bv = b_attn[0:C], b_attn[C:2 * C], b_attn[2 * C:3 * C]
    umsk = np.triu(np.ones((128, 128), dtype=np.float32), k=1)
    nid = -240.0 * np.eye(128, dtype=np.float32)
    in_maps = []
    for core in range(NCORES):
        b, g = divmod(core, 4)
        cs = slice(256 * g, 256 * (g + 1))
        # x^T -> [128, c-pair s, slot j, t]
        x4 = x[b].T.reshape(4, 2, 128, T).transpose(2, 0, 1, 3)
        # chunks m: [q-pair0, q-pair1, k-pair0, k-pair1], natural col order
        wqk_cols = np.concatenate([wq[:, cs], wk[:, cs]], axis=1)
        wqk4 = wqk_cols.reshape(4, 2, 128, 512).transpose(2, 0, 1, 3)
        bqk_ = np.concatenate([bq[cs], bk[cs]]).reshape(4, 128).T.copy()
        wv4 = wv[:, cs].reshape(4, 2, 128, 256).transpose(2, 0, 1, 3)
        wp2 = w_proj[cs, :].reshape(2, 128, 1024).transpose(1, 0, 2)
        in_maps.append({
            "x4": _cast(x4, XW8),
            "wqk": _cast(wqk4, XW8),
            "wv": _cast(wv4, XW8),
            "wp": _cast(wp2, YP8),
            "bqk": np.ascontiguousarray(bqk_, dtype=np.float32),
            "bvb": _bf(np.concatenate([
                np.broadcast_to(bv[cs][None, :], (128, 256)),
                np.ones((128, 64), dtype=np.float32)], axis=1)),
            "umsk": _bf(umsk),
            "nid": _bf(nid),
            "idn": _bf(np.eye(128, dtype=np.float32)),
        })
    return in_maps


def gather_output(results, b_proj):
    b_proj = np.asarray(b_proj, dtype=np.float32)
    out = np.empty((B, T, C), dtype=np.float32)
    for b in range(B):
        acc = results[4 * b]["out"].astype(np.float32)
        for g in range(1, 4):
            acc = acc + results[4 * b + g]["out"].astype(np.float32)
        out[b] = acc + b_proj[None, :]
    return out


def kernel(x, w_attn, b_attn, w_proj, b_proj):
    nc = _get_program()
    in_maps = make_in_maps(x, w_attn, b_attn, w_proj, b_proj)
    res = run_bass_kernel_spmd(nc, in_maps, core_ids=list(range(NCORES)))
    return gather_output(res.results, b_proj)
